# revision 1
# baseline (speedup 1.0000x reference)
"""Trainium2 kernel for CustomFullyConnectedLayer (topk_masking).

Math: out = x @ W.T with W[r, c] = a[(r-c) % n] * V[(r-c) % n, c], where
a = soft top-k mask of alpha (Dykstra projection onto the capped simplex).
a is exactly zero off the top-k, so W has ~K_TOP+eps nonzero pseudo-diagonals,
but the dense matmul on the PE array is still far faster than any sparse
formulation on the vector engines, so we run it dense in bf16.

Sharding: data-parallel over batch — each of the 8 cores computes a
1024-row slice of the output. W.T (bf16, 18.9 MB) is replicated and held
fully resident in SBUF; x.T streams in per 128-row batch chunk.

The tiny Dykstra projection (50 iters on a 3072-vector) and the sparse
scatter that builds W.T are done host-side in fp32 numpy; all O(B*N^2)
work runs on the NeuronCores.
"""

import numpy as np
import ml_dtypes

N = 3072
BATCH = 8192
K_TOP = 16
ALPHA_LR = 0.01
NUM_ITER = 50

NCORES = 8
BPC = BATCH // NCORES  # 1024 batch rows per core
P = 128
KT = N // P            # 24 contraction chunks
BT = BPC // P          # 8 batch chunks per core
RF = 512               # matmul free dim (one PSUM bank of fp32)
RT = N // RF           # 6 output-column chunks

_NC_CACHE = {}


def _dykstra_topk(alpha, k=K_TOP, l=ALPHA_LR, num_iter=NUM_ITER):
    """fp32 numpy port of the reference jax Dykstra loop (same op order)."""
    z = (alpha / np.float32(l)).astype(np.float32)
    n = z.shape[0]
    x = z.copy()
    p = np.zeros_like(z)
    q = np.zeros_like(z)
    for _ in range(num_iter):
        y = x + p
        y = y + (k - np.sum(y, dtype=np.float32)) / n
        p = x + p - y
        x = np.clip(y + q, np.float32(0.0), np.float32(1.0))
        q = y + q - x
    return x


def _build_wt_bf16(V, alpha):
    """W.T[c, r] = a[(r-c)%n] * V[(r-c)%n, c], cast to bf16."""
    a = _dykstra_topk(alpha.astype(np.float32))
    n = N
    nz = np.nonzero(a)[0]
    wt = np.zeros((n, n), np.float32)
    c = np.arange(n)
    for i in nz:
        wt[c, (c + i) % n] = np.float32(a[i]) * V[i, :]
    return wt.astype(ml_dtypes.bfloat16)


def _build_nc():
    import concourse.bacc as bacc
    import concourse.mybir as mybir
    import concourse.tile as tile

    nc = bacc.Bacc("TRN2", target_bir_lowering=False, debug=False,
                   num_devices=NCORES)
    xt = nc.dram_tensor("xt", (BT, P, KT, P), mybir.dt.bfloat16,
                        kind="ExternalInput")
    wt = nc.dram_tensor("wt", (N, N), mybir.dt.bfloat16, kind="ExternalInput")
    out = nc.dram_tensor("out", (BPC, N), mybir.dt.float32,
                         kind="ExternalOutput")

    with tile.TileContext(nc) as tc:
        with (
            tc.tile_pool(name="wpool", bufs=1) as wpool,
            tc.tile_pool(name="xpool", bufs=3) as xpool,
            tc.tile_pool(name="opool", bufs=6) as opool,
            tc.tile_pool(name="pspool", bufs=8, space="PSUM") as pspool,
        ):
            # W.T resident in SBUF: 24 chunks of [128, 3072] bf16 (144 KB/partition)
            wts = []
            for k in range(KT):
                wtile = wpool.tile([P, N], mybir.dt.bfloat16,
                                   tag=f"w{k}", name=f"w{k}")
                nc.sync.dma_start(wtile[:], wt.ap()[k * P:(k + 1) * P, :])
                wts.append(wtile)

            for j in range(BT):
                # x.T chunk: [c-in-chunk(part), k-chunk, b] bf16
                xj = xpool.tile([P, KT, P], mybir.dt.bfloat16, name="xj")
                nc.sync.dma_start(xj[:], xt.ap()[j])
                for r in range(RT):
                    ps = pspool.tile([P, RF], mybir.dt.float32, name="ps")
                    for k in range(KT):
                        nc.tensor.matmul(
                            ps[:],
                            xj[:, k, :],
                            wts[k][:, r * RF:(r + 1) * RF],
                            start=(k == 0),
                            stop=(k == KT - 1),
                        )
                    ob = opool.tile([P, RF], mybir.dt.float32, name="ob")
                    nc.vector.tensor_copy(ob[:], ps[:])
                    nc.sync.dma_start(
                        out.ap()[j * P:(j + 1) * P, r * RF:(r + 1) * RF],
                        ob[:],
                    )
    nc.compile()
    return nc


def get_nc():
    if "nc" not in _NC_CACHE:
        _NC_CACHE["nc"] = _build_nc()
    return _NC_CACHE["nc"]


def make_in_maps(x, V, alpha):
    """Host prep: Dykstra + W.T scatter + per-core pre-tiled x.T, all bf16."""
    wt16 = _build_wt_bf16(V, alpha)
    x16 = x.astype(ml_dtypes.bfloat16)
    in_maps = []
    for cid in range(NCORES):
        slab = x16[cid * BPC:(cid + 1) * BPC]          # (1024, 3072)
        # [j, b, kt, p] -> [j, p, kt, b]: xt[j, p, k, b] = x[j*128+b, k*128+p]
        xt = np.ascontiguousarray(
            slab.reshape(BT, P, KT, P).transpose(0, 3, 2, 1))
        in_maps.append({"xt": xt, "wt": wt16})
    return in_maps


def kernel(x, V, alpha):
    from concourse.bass_utils import run_bass_kernel_spmd

    nc = get_nc()
    in_maps = make_in_maps(x, V, alpha)
    res = run_bass_kernel_spmd(nc, in_maps, core_ids=list(range(NCORES)))
    return np.concatenate(
        [res.results[c]["out"] for c in range(NCORES)], axis=0)


# revision 2
# speedup vs baseline: 1.1779x; 1.1779x over previous
"""Trainium2 kernel for CustomFullyConnectedLayer (topk_masking).

Math: out = x @ W.T with W[r, c] = a[(r-c) % n] * V[(r-c) % n, c], where
a = soft top-k mask of alpha (Dykstra projection onto the capped simplex).
a is exactly zero off the top-k, so W has ~K_TOP+eps nonzero pseudo-diagonals,
but the dense matmul on the PE array is still far faster than any sparse
formulation on the vector engines, so we run it dense in bf16.

Sharding: data-parallel over batch — each of the 8 cores computes a
1024-row slice of the output. W.T (bf16, 18.9 MB) is replicated and held
fully resident in SBUF; x.T streams in per 128-row batch chunk.

The tiny Dykstra projection (50 iters on a 3072-vector) and the sparse
scatter that builds W.T are done host-side in fp32 numpy; all O(B*N^2)
work runs on the NeuronCores.
"""

import numpy as np
import ml_dtypes

N = 3072
BATCH = 8192
K_TOP = 16
ALPHA_LR = 0.01
NUM_ITER = 50

NCORES = 8
BPC = BATCH // NCORES  # 1024 batch rows per core
P = 128
KT = N // P            # 24 contraction chunks
BT = BPC // P          # 8 batch chunks per core
RF = 512               # matmul free dim (one PSUM bank of fp32)
RT = N // RF           # 6 output-column chunks

_NC_CACHE = {}


def _dykstra_topk(alpha, k=K_TOP, l=ALPHA_LR, num_iter=NUM_ITER):
    """fp32 numpy port of the reference jax Dykstra loop (same op order)."""
    z = (alpha / np.float32(l)).astype(np.float32)
    n = z.shape[0]
    x = z.copy()
    p = np.zeros_like(z)
    q = np.zeros_like(z)
    for _ in range(num_iter):
        y = x + p
        y = y + (k - np.sum(y, dtype=np.float32)) / n
        p = x + p - y
        x = np.clip(y + q, np.float32(0.0), np.float32(1.0))
        q = y + q - x
    return x


def _build_wt_bf16(V, alpha):
    """W.T[c, r] = a[(r-c)%n] * V[(r-c)%n, c], cast to bf16."""
    a = _dykstra_topk(alpha.astype(np.float32))
    n = N
    nz = np.nonzero(a)[0]
    wt = np.zeros((n, n), np.float32)
    c = np.arange(n)
    for i in nz:
        wt[c, (c + i) % n] = np.float32(a[i]) * V[i, :]
    return wt.astype(ml_dtypes.bfloat16)


def _build_nc():
    import concourse.bacc as bacc
    import concourse.mybir as mybir
    import concourse.tile as tile

    nc = bacc.Bacc("TRN2", target_bir_lowering=False, debug=False,
                   num_devices=NCORES)
    xt = nc.dram_tensor("xt", (BT, P, KT, P), mybir.dt.bfloat16,
                        kind="ExternalInput")
    wt = nc.dram_tensor("wt", (N, N), mybir.dt.bfloat16, kind="ExternalInput")
    out = nc.dram_tensor("out", (BPC, N), mybir.dt.float32,
                         kind="ExternalOutput")

    with tile.TileContext(nc) as tc:
        with (
            tc.tile_pool(name="wpool", bufs=1) as wpool,
            tc.tile_pool(name="xpool", bufs=4) as xpool,
            tc.tile_pool(name="opool", bufs=6) as opool,
            tc.tile_pool(name="pspool", bufs=8, space="PSUM") as pspool,
        ):
            def store(j, r, ps):
                ob = opool.tile([P, RF], mybir.dt.float32, name="ob")
                nc.vector.tensor_copy(ob[:], ps[:])
                nc.sync.dma_start(
                    out.ap()[j * P:(j + 1) * P, r * RF:(r + 1) * RF],
                    ob[:],
                )

            def load_x(j):
                xj = xpool.tile([P, KT, P], mybir.dt.bfloat16, name="xj")
                nc.sync.dma_start(xj[:], xt.ap()[j])
                return xj

            # x for the first two batch chunks up front (phase A needs them)
            xtiles = {0: load_x(0), 1: load_x(1)}

            # W.T resident in SBUF: 24 chunks of [128, 3072] bf16 (144 KB/partition)
            wts = []
            for k in range(KT):
                wtile = wpool.tile([P, N], mybir.dt.bfloat16,
                                   tag=f"w{k}", name=f"w{k}")
                nc.sync.dma_start(wtile[:], wt.ap()[k * P:(k + 1) * P, :])
                wts.append(wtile)

            # Phase A: 8 concurrent PSUM accumulation groups, k-outer, so the
            # PE consumes each W chunk the moment its DMA lands instead of
            # idling through the ~50us W load.
            groupsA = [(0, r) for r in range(RT)] + [(1, 0), (1, 1)]
            psA = {}
            for (j, r) in groupsA:
                psA[(j, r)] = pspool.tile([P, RF], mybir.dt.float32, name="ps")
            for k in range(KT):
                for (j, r) in groupsA:
                    nc.tensor.matmul(
                        psA[(j, r)][:],
                        xtiles[j][:, k, :],
                        wts[k][:, r * RF:(r + 1) * RF],
                        start=(k == 0),
                        stop=(k == KT - 1),
                    )
            for (j, r) in groupsA:
                store(j, r, psA[(j, r)])

            # Phase B: everything else, k-contiguous per group (W is resident)
            for j in range(1, BT):
                if j >= 2:
                    xtiles[j] = load_x(j)
                for r in range(RT):
                    if (j, r) in psA:
                        continue
                    ps = pspool.tile([P, RF], mybir.dt.float32, name="ps")
                    for k in range(KT):
                        nc.tensor.matmul(
                            ps[:],
                            xtiles[j][:, k, :],
                            wts[k][:, r * RF:(r + 1) * RF],
                            start=(k == 0),
                            stop=(k == KT - 1),
                        )
                    store(j, r, ps)
    nc.compile()
    return nc


def get_nc():
    if "nc" not in _NC_CACHE:
        _NC_CACHE["nc"] = _build_nc()
    return _NC_CACHE["nc"]


def make_in_maps(x, V, alpha):
    """Host prep: Dykstra + W.T scatter + per-core pre-tiled x.T, all bf16."""
    wt16 = _build_wt_bf16(V, alpha)
    x16 = x.astype(ml_dtypes.bfloat16)
    in_maps = []
    for cid in range(NCORES):
        slab = x16[cid * BPC:(cid + 1) * BPC]          # (1024, 3072)
        # [j, b, kt, p] -> [j, p, kt, b]: xt[j, p, k, b] = x[j*128+b, k*128+p]
        xt = np.ascontiguousarray(
            slab.reshape(BT, P, KT, P).transpose(0, 3, 2, 1))
        in_maps.append({"xt": xt, "wt": wt16})
    return in_maps


def kernel(x, V, alpha):
    from concourse.bass_utils import run_bass_kernel_spmd

    nc = get_nc()
    in_maps = make_in_maps(x, V, alpha)
    res = run_bass_kernel_spmd(nc, in_maps, core_ids=list(range(NCORES)))
    return np.concatenate(
        [res.results[c]["out"] for c in range(NCORES)], axis=0)
